# revision 1
# baseline (speedup 1.0000x reference)
"""Trainium2 Bass kernel: batched QP projection (Dykstra fixed point) via an
active-set direct solve. Data parallel: 8 NeuronCores x 16 items each.

Per item (fp64/bf16-faithful validated offline, absmax ~7e-4 vs reference):
  AAt = A A^T + eps I;  Mt ~= inv(AAt) (bf16 Newton-Schulz; preconditioner only)
  z0 = x - A^T h0 with AAt h0 = (A x - b)   (preconditioned Chebyshev)
  4 rounds: sigma = (z<0)&mask
     S = AAt - A_sig A_sig^T  ( = A D A^T + eps I, D = diag(1-sigma) )
     solve S w = t2 - A (D z0)  by Chebyshev (Mt-preconditioned), warm start
     z = z0 + A^T w   (split-bf16 expansion)
  out = x* - A^T h with AAt h = (A x* - b), x* = (1-sigma) z

All linear algebra on device in split-bf16 (hi+lo) 3-pass matmuls with fp32
PSUM accumulate. Host wrapper: shard, mask-first permute, layout transposes,
bf16 hi/lo splits.
"""

import sys

for _p in ("/opt/trn_rl_repo", "/opt/pypackages"):
    if _p not in sys.path:
        sys.path.insert(0, _p)

import numpy as np
import ml_dtypes
from contextlib import ExitStack

import concourse.bass as bass
import concourse.tile as tile
from concourse import mybir, bacc
from concourse.alu_op_type import AluOpType

F32 = mybir.dt.float32
BF16 = mybir.dt.bfloat16

B, m, n = 128, 256, 1024
NCORES = 8
I = B // NCORES      # 16
KT = n // 128        # 8
MT = m // 128        # 2
IM = I * m           # 4096
IN = I * n           # 16384
EPS = 1e-6

N_ROUNDS = 4
RICH = [14, 10, 10, 12]
NS_ITERS = 7
AIN, BIN = 0.8340, 0.2173

_CACHE = {}


def bf_split_np(x):
    x = np.asarray(x, np.float32)
    hi = x.astype(ml_dtypes.bfloat16)
    lo = (x - hi.astype(np.float32)).astype(ml_dtypes.bfloat16)
    return hi, lo


def _cheb_coeffs(l, u, iters):
    th, dl = (u + l) / 2.0, (u - l) / 2.0
    sg = th / dl
    out = []
    rho_prev = None
    for k in range(iters):
        if k == 0:
            out.append((0.0, 1.0 / th))
            rho_prev = 1.0 / sg
        else:
            rho = 1.0 / (2.0 * sg - rho_prev)
            out.append((rho * rho_prev, 2.0 * rho / dl))
            rho_prev = rho
    return out  # (beta_k, gamma_k): w_new = w + beta*(w - wprev) + gamma*z


def _build(n_mk):
    SKT = (n_mk + 127) // 128
    nc = bacc.Bacc("TRN2", target_bir_lowering=False, debug=False, num_devices=NCORES)
    at_hi_d = nc.declare_dram_parameter("at_hi", [KT, 128, IM], BF16, isOutput=False)
    at_lo_d = nc.declare_dram_parameter("at_lo", [KT, 128, IM], BF16, isOutput=False)
    l1_hi_d = nc.declare_dram_parameter("l1_hi", [MT, 128, IN], BF16, isOutput=False)
    l1_lo_d = nc.declare_dram_parameter("l1_lo", [MT, 128, IN], BF16, isOutput=False)
    xz_d = nc.declare_dram_parameter("xz", [128, KT * I], F32, isOutput=False)
    bc_d = nc.declare_dram_parameter("bc", [128, MT * I], F32, isOutput=False)
    m01_d = nc.declare_dram_parameter("m01", [128, KT * I], F32, isOutput=False)
    idl_d = nc.declare_dram_parameter("idl", [MT, 128, IM], BF16, isOutput=False)
    out_d = nc.declare_dram_parameter("out", [I, n], F32, isOutput=True)
    bounce_d = nc.dram_tensor("bounce", [I, n], F32)

    with tile.TileContext(nc) as tc, ExitStack() as ctx:
        nc = tc.nc
        ath_p = ctx.enter_context(tc.tile_pool(name="ath", bufs=1))
        res_p = ctx.enter_context(tc.tile_pool(name="res", bufs=1))
        scr_p = ctx.enter_context(tc.tile_pool(name="scr", bufs=2))
        msk_p = ctx.enter_context(tc.tile_pool(name="msk", bufs=4))
        str_p = ctx.enter_context(tc.tile_pool(name="str", bufs=3))
        vec_p = ctx.enter_context(tc.tile_pool(name="vec", bufs=1))
        row_p = ctx.enter_context(tc.tile_pool(name="row", bufs=1))
        ps_p = ctx.enter_context(tc.tile_pool(name="ps", bufs=2, space=bass.MemorySpace.PSUM))

        AT = [ath_p.tile([128, IM], BF16, name=f"ath{k}", tag=f"ath{k}") for k in range(KT)]
        AAth = [res_p.tile([128, IM], BF16, name=f"aah{k}", tag=f"aah{k}") for k in range(MT)]
        AAtl = [res_p.tile([128, IM], BF16, name=f"aal{k}", tag=f"aal{k}") for k in range(MT)]
        Mh = [res_p.tile([128, IM], BF16, name=f"mh{k}", tag=f"mh{k}") for k in range(MT)]
        IDL = [res_p.tile([128, IM], BF16, name=f"sh{k}", tag=f"sh{k}") for k in range(MT)]  # -> Sh later
        Sl = [res_p.tile([128, IM], BF16, name=f"sl{k}", tag=f"sl{k}") for k in range(MT)]
        Hb = [str_p.tile([128, IM], BF16, name=f"hbc{k}", tag="atlc", bufs=2) for k in range(MT)]  # NS-only; atlc slots free then

        zv = vec_p.tile([128, KT * I], F32, name="zv", tag="zv")
        z0v = vec_p.tile([128, KT * I], F32, name="z0v", tag="z0v")
        uv = vec_p.tile([128, KT * I], F32, name="uv", tag="uv")
        sig = vec_p.tile([128, KT * I], F32, name="sig", tag="sig")
        m01v = vec_p.tile([128, KT * I], F32, name="m01v", tag="m01v")
        xzv = vec_p.tile([128, KT * I], F32, name="xzv", tag="xzv")
        ubh = vec_p.tile([128, KT * I], BF16, name="ubh", tag="ubh")
        ubl = vec_p.tile([128, KT * I], BF16, name="ubl", tag="ubl")
        ztmp = vec_p.tile([128, KT * I], F32, name="ztmp", tag="ztmp")
        bcol = vec_p.tile([128, MT * I], F32, name="bcol", tag="bcol")
        gcol = vec_p.tile([128, MT * I], F32, name="gcol", tag="gcol")
        hcol = vec_p.tile([128, MT * I], F32, name="hcol", tag="hcol")
        wcol = vec_p.tile([128, MT * I], F32, name="wcol", tag="wcol")
        wprev = vec_p.tile([128, MT * I], F32, name="wprev", tag="wprev")
        wtmp = vec_p.tile([128, MT * I], F32, name="wtmp", tag="wtmp")
        t2col = vec_p.tile([128, MT * I], F32, name="t2col", tag="t2col")
        rhsc = vec_p.tile([128, MT * I], F32, name="rhsc", tag="rhsc")
        rcol = vec_p.tile([128, MT * I], F32, name="rcol", tag="rcol")
        mtmp = vec_p.tile([128, MT * I], F32, name="mtmp", tag="mtmp")
        gbh = vec_p.tile([128, MT * I], BF16, name="gbh", tag="gbh")
        gbl = vec_p.tile([128, MT * I], BF16, name="gbl", tag="gbl")

        # ---------------- helpers ----------------
        def split_small(hi, lo, src, tmp):
            nc.vector.tensor_copy(hi[:], src[:])
            nc.vector.tensor_tensor(tmp[:], src[:], hi[:], AluOpType.subtract)
            nc.vector.tensor_copy(lo[:], tmp[:])

        _last_stream = {}

        def _stream_into(tag_key, t, src_ap):
            nc.gpsimd.dma_start(out=t[:], in_=src_ap)
            _last_stream[tag_key] = t
            return t

        def atl_full(kt):
            t = str_p.tile([128, IM], BF16, name="atlc", tag="atlc", bufs=2)
            return _stream_into("atlc", t, at_lo_d[kt])

        def mm_batch(passes, kts, post):
            """out[i] = sum_passes lhsT[i].T @ rhs[i] over kts; psum chunks of
            8 items; post(mt, g0, GI, ps). src "ATL" streams at_lo tiles."""
            GI = 8
            for mt in range(MT):
                for g0 in range(0, I, GI):
                    ps = ps_p.tile([128, 2048], F32, name="psb", tag="psb")
                    npass = len(passes)
                    for ki, kt in enumerate(kts):
                        cache = None
                        for pi, (lhs_t, rhs_t) in enumerate(passes):
                            if lhs_t == "ATL" or rhs_t == "ATL":
                                if cache is None:
                                    cache = atl_full(kt)
                            lt = cache if lhs_t == "ATL" else lhs_t[kt]
                            rt = cache if rhs_t == "ATL" else rhs_t[kt]
                            for gi in range(GI):
                                i = g0 + gi
                                nc.tensor.matmul(
                                    ps[:, gi * m:(gi + 1) * m],
                                    lt[:, i * m + mt * 128: i * m + mt * 128 + 128],
                                    rt[:, i * m:(i + 1) * m],
                                    start=(pi == 0 and ki == 0 and gi % 2 == 0),
                                    stop=(pi == npass - 1 and ki == len(kts) - 1
                                          and gi % 2 == 1),
                                )
                    post(mt, g0, GI, ps)

        def s_build(last):
            """S = AAt - A_sig A_sig^T; Sh (+Sl if last). Mask lhs once per
            (kt,item); both mt psums live."""
            GI = 8
            for g0 in range(0, I, GI):
                pss = [ps_p.tile([128, 2048], F32, name="psb", tag="psb") for _ in range(MT)]
                first = True
                for ki, kt in enumerate(range(SKT)):
                    atl_t = atl_full(kt) if last else None
                    for gi in range(GI):
                        i = g0 + gi
                        mk_hi = msk_p.tile([128, m], BF16, name="mskh", tag="mskh")
                        nc.vector.tensor_scalar(
                            mk_hi[:], AT[kt][:, i * m:(i + 1) * m],
                            sig[:, kt * I + i:kt * I + i + 1], None, AluOpType.mult)
                        if last:
                            mk_lo = msk_p.tile([128, m], BF16, name="mskl", tag="mskl")
                            nc.vector.tensor_scalar(
                                mk_lo[:], atl_t[:, i * m:(i + 1) * m],
                                sig[:, kt * I + i:kt * I + i + 1], None, AluOpType.mult)
                        lst = (ki == SKT - 1 and gi == GI - 1)
                        for mt in range(MT):
                            sl_l = slice(mt * 128, mt * 128 + 128)
                            # pass hi*hi
                            nc.tensor.matmul(
                                pss[mt][:, gi * m:(gi + 1) * m],
                                mk_hi[:, sl_l],
                                AT[kt][:, i * m:(i + 1) * m],
                                start=(ki == 0 and gi % 2 == 0),
                                stop=(ki == SKT - 1 and gi % 2 == 1 and not last))
                            if last:
                                nc.tensor.matmul(
                                    pss[mt][:, gi * m:(gi + 1) * m],
                                    mk_hi[:, sl_l],
                                    atl_t[:, i * m:(i + 1) * m],
                                    start=False, stop=False)
                                nc.tensor.matmul(
                                    pss[mt][:, gi * m:(gi + 1) * m],
                                    mk_lo[:, sl_l],
                                    AT[kt][:, i * m:(i + 1) * m],
                                    start=False, stop=(ki == SKT - 1 and gi % 2 == 1))
                for mt in range(MT):
                    sl_c = slice(g0 * m, (g0 + GI) * m)
                    tmp = scr_p.tile([128, 2048], F32, name="chunk", tag="chunk")
                    nc.vector.tensor_copy(tmp[:], AAtl[mt][:, sl_c])
                    nc.vector.tensor_tensor(tmp[:], tmp[:], pss[mt][:], AluOpType.subtract)
                    nc.vector.tensor_tensor(tmp[:], AAth[mt][:, sl_c], tmp[:], AluOpType.add)
                    nc.vector.tensor_copy(Sh[mt][:, sl_c], tmp[:])
                    if last:
                        nc.vector.tensor_tensor(tmp[:], tmp[:], Sh[mt][:, sl_c],
                                                AluOpType.subtract)
                        nc.vector.tensor_copy(Sl[mt][:, sl_c], tmp[:])

        def row_scatter(ps, c0, CH, N):
            rowc = row_p.tile([16, 2048], F32, name="rowc", tag="rowc")
            half = CH * N // 2
            nc.vector.tensor_copy(rowc[:, 0:half], ps[0:I, 0:half])
            nc.scalar.copy(rowc[:, half:CH * N], ps[0:I, half:CH * N])
            for ci in range(CH):
                i = c0 + ci
                nc.sync.dma_start(out=bounce_d[i, 0:N],
                                  in_=rowc[i:i + 1, ci * N:(ci + 1) * N])

        def col_gather(col_out, N, nt):
            for i in range(I):
                src = bounce_d[i, 0:N].rearrange("(t p) -> p t", p=128)
                dst = col_out.rearrange("p (t i) -> p t i", i=I)[:, 0:nt, i]
                nc.sync.dma_start(out=dst, in_=src)

        def mv_batch(col_out, passes, N, nt_out, streams=None):
            """passes: (wt_list, src) with src tiles-list or stream name.
            streams: {name: loader(ki, c0, CH) -> AP}. Chunk cache per (c0,ki)."""
            CH = 2048 // N
            for c0 in range(0, I, CH):
                ps = ps_p.tile([128, 2048], F32, name="psb", tag="psb")
                cache = {}
                npass = len(passes)
                nk = len(passes[0][0])
                for ki in range(nk):
                    for pi, (wts, src) in enumerate(passes):
                        if isinstance(src, str):
                            key = (src, ki)
                            if key not in cache:
                                cache[key] = streams[src](ki, c0, CH)
                            cols = cache[key]
                        else:
                            cols = src[ki][:, c0 * N:(c0 + CH) * N]
                        for q0 in range(0, CH * N, 512):
                            nc.tensor.matmul(
                                ps[0:I, q0:q0 + 512],
                                wts[ki],
                                cols[:, q0:q0 + 512],
                                start=(pi == 0 and ki == 0),
                                stop=(pi == npass - 1 and ki == nk - 1),
                            )
                row_scatter(ps, c0, CH, N)
            col_gather(col_out, N, nt_out)

        def wt_of(t):
            nt = t.shape[1] // I
            return [t[:, k * I:(k + 1) * I] for k in range(nt)]

        def l1hi_stream(ki, c0, CH):
            t = str_p.tile([128, 2048], BF16, name="l1c", tag="l1c", bufs=2)
            return _stream_into("l1c", t, l1_hi_d[ki][:, c0 * n:(c0 + CH) * n])[:]

        def l1lo_stream(ki, c0, CH):
            t = str_p.tile([128, 2048], BF16, name="l1c", tag="l1c", bufs=2)
            return _stream_into("l1c", t, l1_lo_d[ki][:, c0 * n:(c0 + CH) * n])[:]

        STREAMS = {"L1H": l1hi_stream, "L1L": l1lo_stream}

        def msp_mv(col_out, vh, vl, Qh, Ql, split):
            ps = [(wt_of(vh), Qh)]
            if split:
                ps = [(wt_of(vh), Qh), (wt_of(vl), Qh), (wt_of(vh), Ql)]
            mv_batch(col_out, ps, m, MT)

        def atlo_chunk(ki, c0, CH):
            t = str_p.tile([128, 2048], BF16, name="l1c", tag="l1c", bufs=2)
            return _stream_into("l1c", t, at_lo_d[ki][:, c0 * m:(c0 + CH) * m])[:]

        def dn_mv(col_out, vh, vl, split):
            ps = [(wt_of(vh), AT)]
            if split:
                ps = [(wt_of(vh), AT), (wt_of(vl), AT), (wt_of(vh), "ATLC")]
            mv_batch(col_out, ps, m, MT, streams={"ATLC": atlo_chunk})

        def up_mv(col_out, vh, vl, split):
            ps = [(wt_of(vh), "L1H"), (wt_of(vl), "L1H")]
            if split:
                ps = ps + [(wt_of(vh), "L1L")]
            mv_batch(col_out, ps, n, KT, streams=STREAMS)

        def cheb(Qh, Ql, iters, l, u, use_split, warm):
            if not warm:
                nc.gpsimd.memset(wcol[:], 0.0)
                nc.gpsimd.memset(wprev[:], 0.0)
            for k, (beta, gamma) in enumerate(_cheb_coeffs(l, u, iters)):
                split_small(gbh, gbl, wcol, mtmp)
                msp_mv(rcol, gbh, gbl, Qh, Ql, use_split)
                nc.vector.tensor_tensor(rcol[:], rhsc[:], rcol[:], AluOpType.subtract)
                nc.vector.tensor_copy(gbh[:], rcol[:])
                msp_mv(mtmp, gbh, None, Mh, None, False)
                if k == 0 and not warm:
                    nc.vector.tensor_scalar(wcol[:], mtmp[:], gamma, None, AluOpType.mult)
                    nc.vector.tensor_copy(wprev[:], wcol[:])
                else:
                    nc.vector.tensor_tensor(wtmp[:], wcol[:], wprev[:], AluOpType.subtract)
                    nc.vector.tensor_copy(wprev[:], wcol[:])
                    nc.vector.scalar_tensor_tensor(wtmp[:], wtmp[:], beta, wcol[:],
                                                   AluOpType.mult, AluOpType.add)
                    nc.vector.scalar_tensor_tensor(wcol[:], mtmp[:], gamma, wtmp[:],
                                                   AluOpType.mult, AluOpType.add)

        # ============ loads ============
        nc.sync.dma_start(out=xzv[:], in_=xz_d[:])
        nc.sync.dma_start(out=bcol[:], in_=bc_d[:])
        nc.sync.dma_start(out=m01v[:], in_=m01_d[:])
        for mt in range(MT):
            nc.sync.dma_start(out=IDL[mt][:], in_=idl_d[mt])
        for kt in range(KT):
            nc.sync.dma_start(out=AT[kt][:], in_=at_hi_d[kt])

        # ============ AAt = A A^T + eps I (split) ============
        def post_aat(mt, g0, GI, ps):
            sl_c = slice(g0 * m, (g0 + GI) * m)
            tmp = scr_p.tile([128, 2048], F32, name="chunk", tag="chunk")
            nc.vector.scalar_tensor_tensor(tmp[:], IDL[mt][:, sl_c], EPS, ps[:],
                                           AluOpType.mult, AluOpType.add)
            nc.vector.tensor_copy(AAth[mt][:, sl_c], tmp[:])
            nc.vector.tensor_tensor(tmp[:], tmp[:], AAth[mt][:, sl_c], AluOpType.subtract)
            nc.vector.tensor_copy(AAtl[mt][:, sl_c], tmp[:])
        mm_batch([(AT, AT), (AT, "ATL"), ("ATL", AT)], range(KT), post_aat)

        # ============ Mt: Newton-Schulz bf16 ============
        assert NS_ITERS % 2 == 1
        Xbufs = [Sl, Mh]   # ping-pong; X0 -> Sl, final (odd) lands in Mh
        for mt in range(MT):
            for c0 in range(0, IM, 2048):
                tmp = scr_p.tile([128, 2048], F32, name="chunk", tag="chunk")
                nc.vector.tensor_scalar(tmp[:], AAth[mt][:, c0:c0 + 2048], -BIN, None,
                                        AluOpType.mult)
                nc.vector.scalar_tensor_tensor(tmp[:], IDL[mt][:, c0:c0 + 2048], AIN,
                                               tmp[:], AluOpType.mult, AluOpType.add)
                nc.vector.tensor_copy(Xbufs[0][mt][:, c0:c0 + 2048], tmp[:])
        for it in range(NS_ITERS):
            Xcur = Xbufs[it % 2]
            Xnxt = Xbufs[(it + 1) % 2]
            def post_p1(mt, g0, GI, ps):
                nc.vector.tensor_copy(Hb[mt][:, g0 * m:(g0 + GI) * m], ps[:])
            mm_batch([(AAth, Xcur)], range(MT), post_p1)
            def post_p2(mt, g0, GI, ps, Xc=Xcur, Xn=Xnxt):
                sl_c = slice(g0 * m, (g0 + GI) * m)
                nc.vector.scalar_tensor_tensor(Xn[mt][:, sl_c], Xc[mt][:, sl_c], 2.0,
                                               ps[:], AluOpType.mult, AluOpType.subtract)
            mm_batch([(Xcur, Hb)], range(MT), post_p2)

        # ============ z0, t2 ============
        split_small(ubh, ubl, xzv, ztmp)
        dn_mv(gcol, ubh, ubl, True)
        nc.vector.tensor_tensor(gcol[:], gcol[:], bcol[:], AluOpType.subtract)
        nc.vector.tensor_copy(rhsc[:], gcol[:])
        cheb(AAth, AAtl, 5, 0.80, 1.25, True, warm=False)
        nc.vector.tensor_copy(hcol[:], wcol[:])
        nc.vector.tensor_copy(rhsc[:], bcol[:])
        cheb(AAth, AAtl, 5, 0.80, 1.25, True, warm=False)
        split_small(gbh, gbl, wcol, mtmp)
        msp_mv(t2col, gbh, gbl, AAth, AAtl, True)
        nc.vector.scalar_tensor_tensor(t2col[:], wcol[:], EPS, t2col[:],
                                       AluOpType.mult, AluOpType.add)
        split_small(gbh, gbl, hcol, mtmp)
        up_mv(z0v, gbh, gbl, True)
        nc.vector.tensor_tensor(z0v[:], xzv[:], z0v[:], AluOpType.subtract)

        # ============ rounds ============
        nc.vector.tensor_copy(zv[:], z0v[:])
        Sh = IDL  # identity dead from here; tags sh0/sh1 reused as Sh
        for r in range(N_ROUNDS):
            last = r == N_ROUNDS - 1
            nc.vector.tensor_scalar(sig[:], zv[:], 0.0, None, AluOpType.is_lt)
            nc.vector.tensor_tensor(sig[:], sig[:], m01v[:], AluOpType.mult)
            s_build(last)
            nc.vector.scalar_tensor_tensor(uv[:], sig[:], 0.0, z0v[:],
                                           AluOpType.is_equal, AluOpType.mult)
            split_small(ubh, ubl, uv, ztmp)
            dn_mv(rhsc, ubh, ubl, last)
            nc.vector.tensor_tensor(rhsc[:], t2col[:], rhsc[:], AluOpType.subtract)
            cheb(Sh, Sl, RICH[r], 0.07, 1.30, use_split=last, warm=(r > 0))
            split_small(gbh, gbl, wcol, mtmp)
            up_mv(zv, gbh, gbl, last)
            nc.vector.tensor_tensor(zv[:], z0v[:], zv[:], AluOpType.add)

        # ============ final ============
        nc.vector.tensor_scalar(sig[:], zv[:], 0.0, None, AluOpType.is_lt)
        nc.vector.tensor_tensor(sig[:], sig[:], m01v[:], AluOpType.mult)
        nc.vector.scalar_tensor_tensor(uv[:], sig[:], 0.0, zv[:],
                                       AluOpType.is_equal, AluOpType.mult)
        split_small(ubh, ubl, uv, ztmp)
        dn_mv(gcol, ubh, ubl, True)
        nc.vector.tensor_tensor(gcol[:], gcol[:], bcol[:], AluOpType.subtract)
        nc.vector.tensor_copy(rhsc[:], gcol[:])
        cheb(AAth, AAtl, 5, 0.80, 1.25, True, warm=False)
        split_small(gbh, gbl, wcol, mtmp)
        up_mv(ztmp, gbh, gbl, True)
        nc.vector.tensor_tensor(ztmp[:], uv[:], ztmp[:], AluOpType.subtract)
        for i in range(I):
            src = ztmp.rearrange("p (t i) -> p t i", i=I)[:, :, i]
            dst = out_d[i, :].rearrange("(t p) -> p t", p=128)
            nc.sync.dma_start(out=dst, in_=src)

    nc.compile()
    return nc


def _prep_core(Ap, xp, bp, m01p):
    at = np.ascontiguousarray(Ap.transpose(2, 0, 1)).reshape(KT, 128, IM)
    l1 = np.ascontiguousarray(Ap.transpose(1, 0, 2)).reshape(MT, 128, IN)
    at_hi, at_lo = bf_split_np(at)
    l1_hi, l1_lo = bf_split_np(l1)
    xz = np.ascontiguousarray(xp.T.reshape(KT, 128, I).transpose(1, 0, 2)).reshape(128, KT * I)
    bc = np.ascontiguousarray(bp.T.reshape(MT, 128, I).transpose(1, 0, 2)).reshape(128, MT * I)
    m01 = np.ascontiguousarray(
        np.broadcast_to(m01p.reshape(KT, 128, 1), (KT, 128, I)).transpose(1, 0, 2)
    ).reshape(128, KT * I).astype(np.float32)
    idl = np.zeros((MT, 128, I, m), dtype=np.float32)
    for mt in range(MT):
        for p in range(128):
            idl[mt, p, :, mt * 128 + p] = 1.0
    idl_bf = idl.reshape(MT, 128, IM).astype(ml_dtypes.bfloat16)
    return dict(at_hi=at_hi, at_lo=at_lo, l1_hi=l1_hi, l1_lo=l1_lo,
                xz=np.ascontiguousarray(xz, dtype=np.float32),
                bc=np.ascontiguousarray(bc, dtype=np.float32),
                m01=m01, idl=idl_bf)


_SHIMMED = False


def _fix_cc_flags():
    """Route static DMAs through SP so multi-wait DMAs are legal walrus
    codegen (the embedded-wait form only fits one sync wait)."""
    global _SHIMMED
    try:
        from concourse.compiler_utils import get_compiler_flags, set_compiler_flags
        flags = get_compiler_flags()
        nf = [f.replace("--assign-static-dmas-to-sp=false",
                        "--assign-static-dmas-to-sp=true") for f in flags]
        if nf != flags:
            set_compiler_flags(nf)
    except Exception:
        pass
    if not _SHIMMED:
        import concourse.bass_utils as BU
        orig = BU.run_command

        def patched(cmd, *a, **k):
            if isinstance(cmd, (list, tuple)):
                cmd = [str(c).replace("--assign-static-dmas-to-sp=false",
                                      "--assign-static-dmas-to-sp=true") for c in cmd]
            return orig(cmd, *a, **k)

        BU.run_command = patched
        _SHIMMED = True


def kernel(x, b, A, nonnegative_mask):
    from concourse.bass_utils import run_bass_kernel_spmd
    _fix_cc_flags()
    x = np.asarray(x, dtype=np.float32)
    b = np.asarray(b, dtype=np.float32)
    A = np.asarray(A, dtype=np.float32)
    mk = np.asarray(nonnegative_mask).astype(bool)

    perm = np.argsort(~mk, kind="stable")
    inv = np.argsort(perm, kind="stable")
    n_mk = int(mk.sum())
    Ap = A[:, :, perm]
    xp = x[:, perm]
    m01p = np.zeros(n, np.float32)
    m01p[:n_mk] = 1.0

    if n_mk not in _CACHE:
        _CACHE[n_mk] = _build(n_mk)
    nc = _CACHE[n_mk]

    in_maps = []
    for c in range(NCORES):
        s = slice(c * I, (c + 1) * I)
        in_maps.append(_prep_core(Ap[s], xp[s], b[s], m01p))
    res = run_bass_kernel_spmd(nc, in_maps, core_ids=list(range(NCORES)))
    out_p = np.concatenate([r["out"] for r in res.results], axis=0)
    return np.ascontiguousarray(out_p[:, inv]).astype(np.float32)



# revision 3
# speedup vs baseline: 1.0473x; 1.0473x over previous
"""Trainium2 Bass kernel v2: batched QP projection via active-set direct solve.
Data parallel: 8 NeuronCores x 16 items.

bf16-only (no hi/lo splits), no Newton-Schulz preconditioner: raw Chebyshev
on AAt / S with measured spectral bounds. All matvecs orientation-B
(matrix-stationary [128,128] tiles, vector-moving [128,1] cols) so results
land directly in column layout — no DRAM bounce, no transposes. A resident
in SBUF in both layouts (at: n-partition, l1: m-partition). Sparse round
rhs via the c0 residual trick; masked-only up-passes except last round.

Validated offline (sim2.py, all 128 items): rel err ~2.9e-3 vs gate 2e-2.
"""

import sys

for _p in ("/opt/trn_rl_repo", "/opt/pypackages"):
    if _p not in sys.path:
        sys.path.insert(0, _p)

import numpy as np
import ml_dtypes
from contextlib import ExitStack

import concourse.bass as bass
import concourse.tile as tile
from concourse import mybir, bacc
from concourse.alu_op_type import AluOpType

F32 = mybir.dt.float32
BF16 = mybir.dt.bfloat16

B, m, n = 128, 256, 1024
NCORES = 8
I = B // NCORES      # 16
KT = n // 128        # 8
MT = m // 128        # 2
IM = I * m           # 4096
IN = I * n           # 16384

# solver schedule (validated in sim2.py: rel err ~3.0e-3 vs 2e-2 gate)
INIT_IT = 3
FIN_IT = 3
ROUNDS = 3
R_IT = [5, 4, 4]
AAT_B = (0.24, 2.28)
S_B = [(0.12, 2.2), (0.16, 2.2), (0.16, 2.2)]

_CACHE = {}


def _cheb_coeffs(l, u, iters):
    th, dl = (u + l) / 2.0, (u - l) / 2.0
    sg = th / dl
    out = []
    rho_prev = None
    for k in range(iters):
        if k == 0:
            out.append((0.0, 1.0 / th))
            rho_prev = 1.0 / sg
        else:
            rho = 1.0 / (2.0 * sg - rho_prev)
            out.append((rho * rho_prev, 2.0 * rho / dl))
            rho_prev = rho
    return out  # (beta_k, gamma_k)


def _build(n_mk):
    SKT = (n_mk + 127) // 128
    nc = bacc.Bacc("TRN2", target_bir_lowering=False, debug=False, num_devices=NCORES)
    at_d = nc.declare_dram_parameter("at_hi", [KT, 128, IM], BF16, isOutput=False)
    l1_d = nc.declare_dram_parameter("l1_hi", [MT, 128, IN], BF16, isOutput=False)
    xz_d = nc.declare_dram_parameter("xz", [128, KT * I], F32, isOutput=False)
    bc_d = nc.declare_dram_parameter("bc", [128, MT * I], F32, isOutput=False)
    m01_d = nc.declare_dram_parameter("m01", [128, KT * I], F32, isOutput=False)
    out_d = nc.declare_dram_parameter("out", [I, n], F32, isOutput=True)

    with tile.TileContext(nc) as tc, ExitStack() as ctx:
        nc = tc.nc
        big_p = ctx.enter_context(tc.tile_pool(name="big", bufs=1))
        vec_p = ctx.enter_context(tc.tile_pool(name="vec", bufs=1))
        msk_p = ctx.enter_context(tc.tile_pool(name="msk", bufs=8))
        scr2_p = ctx.enter_context(tc.tile_pool(name="scr2", bufs=2))
        mm_ps = ctx.enter_context(tc.tile_pool(name="mmps", bufs=4,
                                               space=bass.MemorySpace.PSUM))
        sv_ps = ctx.enter_context(tc.tile_pool(name="svps", bufs=2,
                                               space=bass.MemorySpace.PSUM))
        up_ps = ctx.enter_context(tc.tile_pool(name="upps", bufs=2,
                                               space=bass.MemorySpace.PSUM))

        AT = [big_p.tile([128, IM], BF16, name=f"at{k}", tag=f"at{k}")
              for k in range(KT)]
        L1 = [big_p.tile([128, IN], BF16, name=f"l1{k}", tag=f"l1{k}")
              for k in range(MT)]
        AAth = [big_p.tile([128, IM], BF16, name=f"aa{k}", tag=f"aa{k}")
                for k in range(MT)]
        Sh = [big_p.tile([128, IM], BF16, name=f"sh{k}", tag=f"sh{k}")
              for k in range(MT)]

        xzv = vec_p.tile([128, KT * I], F32, name="xzv", tag="xzv")
        m01v = vec_p.tile([128, KT * I], F32, name="m01v", tag="m01v")
        z0v = vec_p.tile([128, KT * I], F32, name="z0v", tag="z0v")
        zv = vec_p.tile([128, KT * I], F32, name="zv", tag="zv")
        sigv = vec_p.tile([128, KT * I], F32, name="sigv", tag="sigv")
        uv = vec_p.tile([128, KT * I], F32, name="uv", tag="uv")
        ztmp = vec_p.tile([128, KT * I], F32, name="ztmp", tag="ztmp")
        ubf = vec_p.tile([128, KT * I], BF16, name="ubf", tag="ubf")
        sgb = vec_p.tile([128, KT * I], BF16, name="sgb", tag="sgb")

        bcol = vec_p.tile([128, MT * I], F32, name="bcol", tag="bcol")
        gcol = vec_p.tile([128, MT * I], F32, name="gcol", tag="gcol")
        rhsc = vec_p.tile([128, MT * I], F32, name="rhsc", tag="rhsc")
        rcol = vec_p.tile([128, MT * I], F32, name="rcol", tag="rcol")
        c0col = vec_p.tile([128, MT * I], F32, name="c0col", tag="c0col")
        wcol = vec_p.tile([128, MT * I], F32, name="wcol", tag="wcol")
        wprev = vec_p.tile([128, MT * I], F32, name="wprev", tag="wprev")
        wtmp = vec_p.tile([128, MT * I], F32, name="wtmp", tag="wtmp")
        wb = vec_p.tile([128, MT * I], BF16, name="wb", tag="wb")

        # ---------------- matvec helpers (orientation B) ----------------
        def mv_m(Mt, wbt):
            """m-space apply: ps[:, mo*I+i] = sum_mi Mt[mi][:,i*m+mo*128:+128].T
            @ wbt[:, mi*I+i]. Returns psum tile [128, MT*I]."""
            ps = sv_ps.tile([128, 512], F32, name="svp", tag="svp")
            for i in range(I):
                for mo in range(MT):
                    c = mo * I + i
                    for mi in range(MT):
                        nc.tensor.matmul(
                            ps[:, c:c + 1],
                            Mt[mi][:, i * m + mo * 128: i * m + mo * 128 + 128],
                            wbt[:, mi * I + i: mi * I + i + 1],
                            start=(i == 0 and mo == 0 and mi == 0),
                            stop=(i == I - 1 and mo == MT - 1 and mi == MT - 1))
            return ps

        def dn(ubt, nk):
            """A v: n->m. ubt [128, KT*I] bf16; contracts kt < nk."""
            ps = sv_ps.tile([128, 512], F32, name="svp", tag="svp")
            for i in range(I):
                for mo in range(MT):
                    c = mo * I + i
                    for kt in range(nk):
                        nc.tensor.matmul(
                            ps[:, c:c + 1],
                            AT[kt][:, i * m + mo * 128: i * m + mo * 128 + 128],
                            ubt[:, kt * I + i: kt * I + i + 1],
                            start=(i == 0 and mo == 0 and kt == 0),
                            stop=(i == I - 1 and mo == MT - 1 and kt == nk - 1))
            return ps

        def up(wbt, nt):
            """A^T w: m->n (first nt n-tiles). Returns psum [128, KT*I]."""
            ps = up_ps.tile([128, 512], F32, name="upp", tag="upp")
            for t in range(nt):
                for i in range(I):
                    c = t * I + i
                    for mi in range(MT):
                        nc.tensor.matmul(
                            ps[:, c:c + 1],
                            L1[mi][:, i * n + t * 128: i * n + t * 128 + 128],
                            wbt[:, mi * I + i: mi * I + i + 1],
                            start=(i == 0 and t == 0 and mi == 0),
                            stop=(i == I - 1 and t == nt - 1 and mi == MT - 1))
            return ps

        def cheb(Mt, iters, l, u, warm):
            if not warm:
                nc.gpsimd.memset(wcol[:], 0.0)
                nc.gpsimd.memset(wprev[:], 0.0)
            else:
                nc.vector.tensor_copy(wprev[:], wcol[:])
            for k, (beta, gamma) in enumerate(_cheb_coeffs(l, u, iters)):
                nc.vector.tensor_copy(wb[:], wcol[:])
                ps = mv_m(Mt, wb)
                nc.vector.tensor_tensor(rcol[:], rhsc[:], ps[:, 0:MT * I],
                                        AluOpType.subtract)
                if k == 0 and not warm:
                    nc.vector.tensor_scalar(wcol[:], rcol[:], gamma, None,
                                            AluOpType.mult)
                else:
                    nc.vector.tensor_tensor(wtmp[:], wcol[:], wprev[:],
                                            AluOpType.subtract)
                    nc.vector.tensor_copy(wprev[:], wcol[:])
                    nc.vector.scalar_tensor_tensor(wtmp[:], wtmp[:], beta, wcol[:],
                                                   AluOpType.mult, AluOpType.add)
                    nc.vector.scalar_tensor_tensor(wcol[:], rcol[:], gamma, wtmp[:],
                                                   AluOpType.mult, AluOpType.add)

        # ================= loads =================
        # spread across 4 engine DMA queues so at tiles land fast; l1 is not
        # needed until the init up-pass so it queues behind at on two queues
        nc.sync.dma_start(out=xzv[:], in_=xz_d[:])
        nc.sync.dma_start(out=bcol[:], in_=bc_d[:])
        nc.sync.dma_start(out=m01v[:], in_=m01_d[:])
        qs = [nc.sync, nc.scalar, nc.gpsimd]
        for kt in range(KT):
            qs[kt % 3].dma_start(out=AT[kt][:], in_=at_d[kt])
        # l1 split into column chunks, balanced across the 3 rings
        # (ring2 carries only 2 at tiles, so it gets the most l1)
        l1_chunks = [(mt, c) for mt in range(MT) for c in range(3)]
        ring_of = [2, 0, 1, 2, 0, 1]
        ch_n = IN // 3 // 512 * 512
        for (mt, c), rq in zip(l1_chunks, ring_of):
            c0 = c * ch_n
            c1 = (c + 1) * ch_n if c < 2 else IN
            qs[rq].dma_start(out=L1[mt][:, c0:c1], in_=l1_d[mt][:, c0:c1])

        # ================= AAt = A A^T (bf16 store) =================
        # two kt-half accumulation groups per chunk so matmuls start after
        # only half the at tiles have landed; halves summed psum+psum -> bf16
        CH = 2
        HK = KT // 2
        for g0 in range(0, I, CH):
            pss = [[mm_ps.tile([128, CH * m], F32, name="mmp", tag="mmp")
                    for _ in range(MT)] for _ in range(2)]
            for h in range(2):
                for kt in range(h * HK, (h + 1) * HK):
                    for gi in range(CH):
                        i = g0 + gi
                        for mo in range(MT):
                            nc.tensor.matmul(
                                pss[h][mo][:, gi * m:(gi + 1) * m],
                                AT[kt][:, i * m + mo * 128: i * m + mo * 128 + 128],
                                AT[kt][:, i * m:(i + 1) * m],
                                start=(kt == h * HK and gi % 2 == 0),
                                stop=(kt == (h + 1) * HK - 1 and gi % 2 == 1))
            for mo in range(MT):
                # walrus rejects dual-PSUM-input DVE ops; bounce one half
                # through SBUF
                hs = scr2_p.tile([128, CH * m], F32, name="hsum", tag="hsum")
                nc.vector.tensor_copy(hs[:], pss[0][mo][:])
                nc.vector.tensor_tensor(AAth[mo][:, g0 * m:(g0 + CH) * m],
                                        hs[:], pss[1][mo][:], AluOpType.add)

        # ================= init affine =================
        nc.vector.tensor_copy(ubf[:], xzv[:])
        ps = dn(ubf, KT)
        nc.vector.tensor_tensor(gcol[:], ps[:, 0:MT * I], bcol[:],
                                AluOpType.subtract)
        nc.vector.tensor_copy(rhsc[:], gcol[:])
        cheb(AAth, INIT_IT, *AAT_B, warm=False)
        # c0 = AAt h0 - g  (= b - A z0)
        nc.vector.tensor_copy(wb[:], wcol[:])
        ps = mv_m(AAth, wb)
        nc.vector.tensor_tensor(c0col[:], ps[:, 0:MT * I], gcol[:],
                                AluOpType.subtract)
        # z0 = x - A^T h0
        psn = up(wb, KT)
        nc.vector.tensor_tensor(z0v[:], xzv[:], psn[:, 0:KT * I],
                                AluOpType.subtract)
        nc.vector.tensor_copy(zv[:], z0v[:])

        # ================= rounds =================
        nc.gpsimd.memset(sigv[:], 0.0)
        for r in range(ROUNDS):
            last = r == ROUNDS - 1
            # sigma = (z < 0) & mask, per masked tile (overlaps prior up-pass)
            for kt in range(SKT):
                sl = slice(kt * I, (kt + 1) * I)
                nc.vector.tensor_scalar(sigv[:, sl], zv[:, sl], 0.0, None,
                                        AluOpType.is_lt)
                nc.vector.tensor_tensor(sigv[:, sl], sigv[:, sl], m01v[:, sl],
                                        AluOpType.mult)
            # S = AAt - (sig*A) A^T
            for g0 in range(0, I, CH):
                pss = [mm_ps.tile([128, CH * m], F32, name="mmp", tag="mmp")
                       for _ in range(MT)]
                for kt in range(SKT):
                    for gi in range(CH):
                        i = g0 + gi
                        mk = msk_p.tile([128, m], BF16, name="mk", tag="mk")
                        nc.vector.tensor_scalar(
                            mk[:], AT[kt][:, i * m:(i + 1) * m],
                            sigv[:, kt * I + i: kt * I + i + 1], None,
                            AluOpType.mult)
                        for mo in range(MT):
                            nc.tensor.matmul(
                                pss[mo][:, gi * m:(gi + 1) * m],
                                mk[:, mo * 128: mo * 128 + 128],
                                AT[kt][:, i * m:(i + 1) * m],
                                start=(kt == 0 and gi % 2 == 0),
                                stop=(kt == SKT - 1 and gi % 2 == 1))
                for mo in range(MT):
                    sl = slice(g0 * m, (g0 + CH) * m)
                    nc.vector.scalar_tensor_tensor(
                        Sh[mo][:, sl], pss[mo][:], -1.0, AAth[mo][:, sl],
                        AluOpType.mult, AluOpType.add)
            # rhs = c0 + A (sig * z0)
            nc.vector.tensor_tensor(ubf[:], sigv[:], z0v[:], AluOpType.mult)
            ps = dn(ubf, SKT)
            nc.vector.tensor_tensor(rhsc[:], c0col[:], ps[:, 0:MT * I],
                                    AluOpType.add)
            cheb(Sh, R_IT[r], *S_B[r], warm=(r > 0))
            # z = z0 + A^T w  (masked tiles only except last round),
            # updated per tile so next round's sigma/masks can start early
            nc.vector.tensor_copy(wb[:], wcol[:])
            nt = KT if last else SKT
            psn = up(wb, nt)
            for t in range(nt):
                sl = slice(t * I, (t + 1) * I)
                nc.vector.tensor_tensor(zv[:, sl], z0v[:, sl], psn[:, sl],
                                        AluOpType.add)

        # ================= final affine on u = D z =================
        nc.vector.tensor_scalar(sigv[:], zv[:], 0.0, None, AluOpType.is_lt)
        nc.vector.tensor_tensor(sigv[:], sigv[:], m01v[:], AluOpType.mult)
        nc.vector.scalar_tensor_tensor(uv[:], sigv[:], 0.0, zv[:],
                                       AluOpType.is_equal, AluOpType.mult)
        nc.vector.tensor_copy(ubf[:], uv[:])
        ps = dn(ubf, KT)
        nc.vector.tensor_tensor(gcol[:], ps[:, 0:MT * I], bcol[:],
                                AluOpType.subtract)
        nc.vector.tensor_copy(rhsc[:], gcol[:])
        cheb(AAth, FIN_IT, *AAT_B, warm=False)
        nc.vector.tensor_copy(wb[:], wcol[:])
        psn = up(wb, KT)
        # out = u - A^T h, permuted item-major in SBUF so one DMA covers it
        nc.vector.tensor_tensor(
            zv.rearrange("p (i t) -> p i t", t=KT),
            uv.rearrange("p (t i) -> p i t", i=I),
            psn[:, 0:KT * I].rearrange("p (t i) -> p i t", i=I),
            AluOpType.subtract)
        src = zv.rearrange("p (i t) -> p i t", t=KT)
        dst = out_d.rearrange("i (t p) -> p i t", p=128)
        nc.sync.dma_start(out=dst, in_=src)

    nc.compile()
    return nc


def _prep_core(Ap, xp, bp, m01p):
    at = np.ascontiguousarray(Ap.transpose(2, 0, 1)).reshape(KT, 128, IM)
    l1 = np.ascontiguousarray(Ap.transpose(1, 0, 2)).reshape(MT, 128, IN)
    at_hi = at.astype(ml_dtypes.bfloat16)
    l1_hi = l1.astype(ml_dtypes.bfloat16)
    xz = np.ascontiguousarray(
        xp.T.reshape(KT, 128, I).transpose(1, 0, 2)).reshape(128, KT * I)
    bc = np.ascontiguousarray(
        bp.T.reshape(MT, 128, I).transpose(1, 0, 2)).reshape(128, MT * I)
    m01 = np.ascontiguousarray(
        np.broadcast_to(m01p.reshape(KT, 128, 1), (KT, 128, I)).transpose(1, 0, 2)
    ).reshape(128, KT * I).astype(np.float32)
    return dict(at_hi=at_hi, l1_hi=l1_hi,
                xz=np.ascontiguousarray(xz, dtype=np.float32),
                bc=np.ascontiguousarray(bc, dtype=np.float32),
                m01=m01)


_SHIMMED = False


def _fix_cc_flags():
    """Route static DMAs through SP so multi-wait DMAs are legal walrus
    codegen (the embedded-wait form only fits one sync wait)."""
    global _SHIMMED
    try:
        from concourse.compiler_utils import get_compiler_flags, set_compiler_flags
        flags = get_compiler_flags()
        nf = [f.replace("--assign-static-dmas-to-sp=false",
                        "--assign-static-dmas-to-sp=true") for f in flags]
        if nf != flags:
            set_compiler_flags(nf)
    except Exception:
        pass
    if not _SHIMMED:
        import concourse.bass_utils as BU
        orig = BU.run_command

        def patched(cmd, *a, **k):
            if isinstance(cmd, (list, tuple)):
                cmd = [str(c).replace("--assign-static-dmas-to-sp=false",
                                      "--assign-static-dmas-to-sp=true") for c in cmd]
            return orig(cmd, *a, **k)

        BU.run_command = patched
        _SHIMMED = True


def kernel(x, b, A, nonnegative_mask):
    from concourse.bass_utils import run_bass_kernel_spmd
    _fix_cc_flags()
    x = np.asarray(x, dtype=np.float32)
    b = np.asarray(b, dtype=np.float32)
    A = np.asarray(A, dtype=np.float32)
    mk = np.asarray(nonnegative_mask).astype(bool)

    perm = np.argsort(~mk, kind="stable")
    inv = np.argsort(perm, kind="stable")
    n_mk = int(mk.sum())
    Ap = A[:, :, perm]
    xp = x[:, perm]
    m01p = np.zeros(n, np.float32)
    m01p[:n_mk] = 1.0

    if n_mk not in _CACHE:
        _CACHE[n_mk] = _build(n_mk)
    nc = _CACHE[n_mk]

    in_maps = []
    for c in range(NCORES):
        s = slice(c * I, (c + 1) * I)
        in_maps.append(_prep_core(Ap[s], xp[s], b[s], m01p))
    res = run_bass_kernel_spmd(nc, in_maps, core_ids=list(range(NCORES)))
    out_p = np.concatenate([r["out"] for r in res.results], axis=0)
    return np.ascontiguousarray(out_p[:, inv]).astype(np.float32)
